# revision 19
# baseline (speedup 1.0000x reference)
"""Trainium2 Bass kernel for capsule attention-routing.

Reference computation (per pixel (b,h,w); 4096 independent problems of
shape [I=32 in-caps, N=32 out-caps, J=16 caps-dim]):
    v[n,j]   = sum_i u[i,n,j]
    cp[i,n]  = sum_j u[i,n,j] * v[n,j] / 4
    c[i,n]   = softmax_n(cp)[i,n] + b[i,n]
    s[n,j]   = sum_i u[i,n,j] * c[i,n]
    out[n,j] = (1 - 1/(exp(|s|_j)+eps)) * s[n,j] / (|s|_j + eps)

Sharding: data-parallel over (batch, h-half): 8 cores x 512 pixels.

Per-core layout: SBUF partitions = (j*8 + il), il = i%8, i = ib*8+il.
All reductions run on the TensorEngine via 0/1/0.25 delta-weight matmuls
(exactly representable -> no weight rounding error):
  v:     contract il (+PSUM-accumulate over ib), broadcast to all rows
  cp:    contract j, pack out partitions (q*32+ib*8+il) with q = p16 pixel blk
  cbc:   broadcast c back over j (K=32 matmuls from c's partition strips)
  s:     contract il (+accum over ib), pack out partitions (q8*16+j), q8 = p8 blk
  norm2: contract j within q8-group, broadcast over group
u streams through PE as float32r; DVE products stored bf16 for 1-cyc/row PE.
Softmax runs without max-subtraction (|cp| <~ 40 is safe in fp32 exp).
EPS=1e-20 is negligible: 1-1/(exp(r)+eps) == 1-exp(-r), 1/(r+eps) == 1/r.
"""

import math
import numpy as np
from contextlib import ExitStack

import concourse.bass as bass
import concourse.bacc as bacc
import concourse.tile as tile
import concourse.mybir as mybir
from concourse.bass_utils import run_bass_kernel_spmd

dt = mybir.dt
AF = mybir.ActivationFunctionType
OP = mybir.AluOpType

B, I, N, J, H, W = 4, 32, 32, 16, 32, 32
HW = H * W
NCORES = 8
PIX = B * HW // NCORES      # 512 pixels per core
BLK = 64                    # pixels per block
P16, P8 = 16, 8
NQ, NQ8 = BLK // P16, BLK // P8   # 4, 8
SCALE = 1.0 / math.sqrt(16.0)     # 0.25

f32, bf16, f32r = dt.float32, dt.bfloat16, dt.float32r


def _build_weight_arrays():
    il_of = np.arange(128) % 8          # partition -> il
    j_of = np.arange(128) // 8          # partition -> j

    # v-pass: out[(j2,il2)] = sum_il u[(j,il)] for j==j2 (broadcast over il2)
    wv = np.zeros((128, 128), np.float32)
    for p_in in range(128):
        for p_out in range(128):
            if j_of[p_in] == j_of[p_out]:
                wv[p_in, p_out] = 1.0

    # c-reduce: 16 blocks k=q*4+ib: out[q*32+ib*8+il] = SCALE*sum_j w[(j,il)]
    wc = np.zeros((128, 16 * 128), np.float32)
    for q in range(4):
        for ib in range(4):
            k = q * 4 + ib
            for p_in in range(128):
                wc[p_in, k * 128 + q * 32 + ib * 8 + il_of[p_in]] = SCALE

    # c-bcast: row strips q*32..q*32+32 each hold the same [32,128] pattern.
    # in strip: row (ib2*8+il2), col-block ib: col (j*8+il): delta(ib2==ib, il2==il)
    wcb = np.zeros((128, 4 * 128), np.float32)
    for q in range(4):
        for ib in range(4):
            for il in range(8):
                for j in range(16):
                    wcb[q * 32 + ib * 8 + il, ib * 128 + j * 8 + il] = 1.0

    # s-reduce: 8 blocks q8: out[q8*16+j2] = sum_il m[(j,il)] with j==j2
    ws = np.zeros((128, 8 * 128), np.float32)
    for q8 in range(8):
        for p_in in range(128):
            ws[p_in, q8 * 128 + q8 * 16 + j_of[p_in]] = 1.0

    # norm2: out[(q8b*16+r)] = sum_j ssq[(q8*16+j)] for q8==q8b
    wn = np.zeros((128, 128), np.float32)
    for p_in in range(128):
        for p_out in range(128):
            if p_in // 16 == p_out // 16:
                wn[p_in, p_out] = 1.0

    return {
        "wv": wv.astype(np.float32),
        "wc": wc.astype(np.dtype(np.float32)).astype("bfloat16")
        if False
        else wc,  # cast handled at upload
        "wcb": wcb,
        "ws": ws,
        "wn": wn,
    }


def _b_tile_array(b_np):
    # b_t[q*32+ib*8+il, n*16+p] = b[0, ib*8+il, n, 0,0,0]
    bt = np.zeros((128, N * P16), np.float32)
    bsl = b_np.reshape(I, N)
    for q in range(4):
        for ib in range(4):
            for il in range(8):
                row = q * 32 + ib * 8 + il
                bt[row, :] = np.repeat(bsl[ib * 8 + il, :], P16)
    return bt


def _emit(ctx: ExitStack, tc: tile.TileContext, aps: dict, pix: int):
    nc = tc.nc
    nblk = pix // BLK
    u_d, o_d = aps["u"], aps["out"]

    # u dram layout: [ib, j, il, n, pix] so (j il) merges into the partition dim
    u_view = u_d.rearrange("ib j il n (blk p) -> ib blk (j il) n p", p=BLK)

    # constant pool
    pconst = ctx.enter_context(tc.tile_pool(name="const", bufs=1))
    wv_t = pconst.tile([128, 128], f32r, tag="wv")
    wc_t = pconst.tile([128, 16 * 128], bf16, tag="wc")
    wcb_t = pconst.tile([32 * 4, 4 * 128], bf16, tag="wcb")
    ws_t = pconst.tile([128, 8 * 128], bf16, tag="ws")
    wn_t = pconst.tile([128, 128], bf16, tag="wn")
    bt_t = pconst.tile([128, N * P16], f32, tag="bt")
    nc.sync.dma_start(wv_t[:], aps["wv"])
    nc.sync.dma_start(wc_t[:], aps["wc"])
    nc.sync.dma_start(wcb_t[:], aps["wcb"])
    nc.sync.dma_start(ws_t[:], aps["ws"])
    nc.sync.dma_start(wn_t[:], aps["wn"])
    nc.sync.dma_start(bt_t[:], aps["bt"])

    pu = ctx.enter_context(tc.tile_pool(name="u", bufs=8))
    pw = ctx.enter_context(tc.tile_pool(name="w", bufs=8))
    pm = ctx.enter_context(tc.tile_pool(name="m", bufs=4))
    pvsb = ctx.enter_context(tc.tile_pool(name="vsb", bufs=2))
    psmall = ctx.enter_context(tc.tile_pool(name="small", bufs=2))
    psq = ctx.enter_context(tc.tile_pool(name="sq", bufs=2))

    pvps = ctx.enter_context(tc.tile_pool(name="vps", bufs=2, space="PSUM"))
    pcps = ctx.enter_context(tc.tile_pool(name="cps", bufs=2, space="PSUM"))
    pcb = ctx.enter_context(tc.tile_pool(name="cb", bufs=1, space="PSUM"))
    pspk = ctx.enter_context(tc.tile_pool(name="spk", bufs=2, space="PSUM"))

    for blk in range(nblk):
        # ---- load u tiles: [(j,il), (n, p64)] ----
        T = []
        for ib in range(4):
            t = pu.tile([128, N * BLK], f32r, tag="T")
            nc.sync.dma_start(
                t[:].rearrange("P (n p) -> P n p", p=BLK), u_view[ib, blk]
            )
            T.append(t)

        # ---- v-pass (PE, f32r): v = sum_i u, broadcast over rows ----
        v_sb = pvsb.tile([128, N * BLK], f32, tag="vsb")
        for st in range(4):
            sl = slice(st * 512, (st + 1) * 512)
            v_ps = pvps.tile([128, 512], f32, tag="vps")
            for ib in range(4):
                nc.tensor.matmul(
                    v_ps[:],
                    wv_t[:],
                    T[ib][:, sl],
                    start=(ib == 0),
                    stop=(ib == 3),
                )
            nc.scalar.copy(v_sb[:, sl], v_ps[:])

        # ---- c-mult (DVE): w = u * v  (store bf16) ----
        Wt = []
        for ib in range(4):
            w = pw.tile([128, N * BLK], bf16, tag="w")
            nc.vector.tensor_tensor(w[:], T[ib][:].bitcast(f32), v_sb[:], op=OP.mult)
            Wt.append(w)

        # ---- c-reduce (PE): cp[(q,ib,il), (n,p16)] = SCALE*sum_j w ----
        c_ps = pcps.tile([128, N * P16], f32, tag="cps")
        c_ps_v = c_ps[:].rearrange("P (n p) -> P n p", p=P16)
        for q in range(4):
            for ib in range(4):
                rhs = Wt[ib][:].rearrange("P (n p) -> P n p", p=BLK)[
                    :, :, q * P16 : (q + 1) * P16
                ]
                nc.tensor.matmul(
                    c_ps_v,
                    wc_t[:, (q * 4 + ib) * 128 : (q * 4 + ib + 1) * 128],
                    rhs,
                    start=(q == 0 and ib == 0),
                    stop=(q == 3 and ib == 3),
                    skip_group_check=True,
                )

        # ---- softmax over n (no max-subtraction; |cp| < ~45) ----
        c_e = psmall.tile([128, N * P16], f32, tag="ce")
        nc.scalar.activation(c_e[:], c_ps[:], AF.Exp)
        z = psmall.tile([128, P16], f32, tag="z")
        nc.vector.tensor_reduce(
            z[:],
            c_e[:].rearrange("P (n p) -> P p n", p=P16),
            axis=mybir.AxisListType.X,
            op=OP.add,
        )
        rz = psmall.tile([128, P16], f32, tag="rz")
        nc.vector.reciprocal(rz[:], z[:])
        c_f = psmall.tile([128, N * P16], f32, tag="cf")
        nc.vector.tensor_tensor(
            c_f[:].rearrange("P (n p) -> P n p", p=P16),
            c_e[:].rearrange("P (n p) -> P n p", p=P16),
            rz[:].rearrange("P (o p) -> P o p", o=1).broadcast_to([128, N, P16]),
            op=OP.mult,
        )
        c_sb = psmall.tile([128, N * P16], bf16, tag="csb")
        nc.vector.tensor_tensor(c_sb[:], c_f[:], bt_t[:], op=OP.add)

        # ---- c-bcast (PE) + s-mult (DVE) + s-reduce (PE) ----
        spk = pspk.tile([128, N * P8], f32, tag="spk")
        spk_v = spk[:].rearrange("P (n p) -> P n p", p=P8)
        first_s = True
        for ib in range(4):
            for q in range(4):
                cb = pcb.tile([128, N * P16], f32, tag="cb")
                nc.tensor.matmul(
                    cb[:].rearrange("P (n p) -> P n p", p=P16),
                    wcb_t[q * 32 : (q + 1) * 32, ib * 128 : (ib + 1) * 128],
                    c_sb[q * 32 : (q + 1) * 32, :].rearrange(
                        "P (n p) -> P n p", p=P16
                    ),
                    start=True,
                    stop=True,
                    skip_group_check=True,
                    tile_position=(q * 32, 0),
                )
                m = pm.tile([128, N * P16], bf16, tag="m")
                nc.vector.tensor_tensor(
                    m[:].rearrange("P (n p) -> P n p", p=P16),
                    T[ib][:].bitcast(f32).rearrange("P (n p) -> P n p", p=BLK)[
                        :, :, q * P16 : (q + 1) * P16
                    ],
                    cb[:].rearrange("P (n p) -> P n p", p=P16),
                    op=OP.mult,
                )
                for k2 in range(2):
                    q8 = 2 * q + k2
                    rhs = m[:].rearrange("P (n p) -> P n p", p=P16)[
                        :, :, k2 * P8 : (k2 + 1) * P8
                    ]
                    nc.tensor.matmul(
                        spk_v,
                        ws_t[:, q8 * 128 : (q8 + 1) * 128],
                        rhs,
                        start=first_s,
                        stop=(ib == 3 and q == 3 and k2 == 1),
                        skip_group_check=True,
                    )
                    first_s = False

        # ---- squash ----
        ssq = psq.tile([128, N * P8], bf16, tag="ssq")
        nc.scalar.activation(ssq[:], spk[:], AF.Square)
        n2 = pcb.tile([128, N * P8], f32, tag="cb")
        nc.tensor.matmul(n2[:], wn_t[:], ssq[:], start=True, stop=True)
        norm = psq.tile([128, N * P8], f32, tag="norm")
        nc.scalar.activation(norm[:], n2[:], AF.Sqrt)
        en = psq.tile([128, N * P8], f32, tag="en")
        nc.scalar.activation(en[:], norm[:], AF.Exp, scale=-1.0)
        rn = psq.tile([128, N * P8], f32, tag="rn")
        nc.vector.reciprocal(rn[:], norm[:])
        g = psq.tile([128, N * P8], f32, tag="g")
        nc.vector.scalar_tensor_tensor(
            g[:], en[:], 1.0, rn[:], op0=OP.subtract, op1=OP.mult
        )  # g = (en - 1) * rn = -(1-en)/norm
        outt = psq.tile([128, N * P8], f32, tag="outt")
        nc.vector.scalar_tensor_tensor(
            outt[:], spk[:], -1.0, g[:], op0=OP.mult, op1=OP.mult
        )  # (-s) * g = s * (1-en)/norm
        # out DRAM layout mirrors the SBUF tile (host reassembles)
        nc.sync.dma_start(o_d[blk], outt[:])


def round_f32r(x):
    """Round fp32 to the PE's fp32r format: 11-bit mantissa (RNE), low 12 bits 0."""
    b = x.view(np.uint32)
    r = (b + np.uint32(0x7FF) + ((b >> np.uint32(12)) & np.uint32(1))) & np.uint32(
        0xFFFFF000
    )
    return r.view(np.float32)


def encode_u(shard):
    """[I, N, J, pix] -> [ib=4, J, il=8, N, pix] device layout, fp32r-rounded."""
    pix = shard.shape[-1]
    a = shard.reshape(4, 8, N, J, pix)
    return round_f32r(np.ascontiguousarray(a.transpose(0, 3, 1, 2, 4)))


def decode_out(arr, pix):
    """[nblk, 128=(q8,j), N*P8] device layout -> [N, J, pix]."""
    nblk = pix // BLK
    a = arr.reshape(nblk, NQ8, J, N, P8)
    return np.ascontiguousarray(a.transpose(3, 2, 0, 1, 4)).reshape(N, J, pix)


_CACHE = {}


def _get_program(pix):
    if pix in _CACHE:
        return _CACHE[pix]
    nc = bacc.Bacc("TRN2", target_bir_lowering=False, debug=False)
    names = {}
    aps = {}
    aps["u"] = nc.dram_tensor(
        "u", [4, J, 8, N, pix], f32r, kind="ExternalInput"
    ).ap()
    wts = _build_weight_arrays()
    aps["wv"] = nc.dram_tensor("wv", [128, 128], f32r, kind="ExternalInput").ap()
    aps["wc"] = nc.dram_tensor("wc", [128, 16 * 128], bf16, kind="ExternalInput").ap()
    aps["wcb"] = nc.dram_tensor("wcb", [128, 4 * 128], bf16, kind="ExternalInput").ap()
    aps["ws"] = nc.dram_tensor("ws", [128, 8 * 128], bf16, kind="ExternalInput").ap()
    aps["wn"] = nc.dram_tensor("wn", [128, 128], bf16, kind="ExternalInput").ap()
    aps["bt"] = nc.dram_tensor("bt", [128, N * P16], f32, kind="ExternalInput").ap()
    aps["out"] = nc.dram_tensor(
        "out", [pix // BLK, 128, N * P8], f32, kind="ExternalOutput"
    ).ap()

    with tile.TileContext(nc) as tc:
        with ExitStack() as ctx:
            _emit(ctx, tc, aps, pix)
    nc.compile()

    _CACHE[pix] = (nc, wts)
    return _CACHE[pix]


def kernel(u: np.ndarray, b: np.ndarray) -> np.ndarray:
    u = np.asarray(u, dtype=np.float32)
    b = np.asarray(b, dtype=np.float32)
    nc, wts = _get_program(PIX)

    import ml_dtypes

    bt = _b_tile_array(b)
    base = {
        "wv": wts["wv"],
        "wc": wts["wc"].astype(ml_dtypes.bfloat16),
        "wcb": wts["wcb"].astype(ml_dtypes.bfloat16),
        "ws": wts["ws"].astype(ml_dtypes.bfloat16),
        "wn": wts["wn"].astype(ml_dtypes.bfloat16),
        "bt": bt,
    }
    in_maps = []
    for c in range(NCORES):
        bb = c // 2
        h0 = 16 * (c % 2)
        shard = u[bb, :, :, :, h0 : h0 + 16, :].reshape(I, N, J, PIX)
        m = dict(base)
        m["u"] = encode_u(shard)
        in_maps.append(m)

    res = run_bass_kernel_spmd(nc, in_maps, core_ids=list(range(NCORES)))
    out = np.zeros((B, N, J, H, W), np.float32)
    for c in range(NCORES):
        bb = c // 2
        h0 = 16 * (c % 2)
        out[bb, :, :, h0 : h0 + 16, :] = decode_out(
            res.results[c]["out"], PIX
        ).reshape(N, J, 16, W)
    return out
